# revision 1
# baseline (speedup 1.0000x reference)
"""Trainium2 Bass kernel for Ernie4.5-VL attention (mRoPE + GQA causal attention).

Sharding: tensor-parallel over heads across 8 cores. Each core computes
2 q heads + its kv head (replicated per core pair): qkv projection
(feature-major), interleaved mRoPE (via a host-side even/odd column
permutation of the q/k weight slices so the rotation becomes two
contiguous partition halves), causal attention with unnormalized softmax
(denominator via an all-ones matmul, normalization after AV), and the
o_proj partial product. Host sums the 8 partial outputs.

All matmuls run in float32r (full PE rate at free-dim 512, ~1.5e-4 rel err).
"""
import numpy as np
import ml_dtypes
from contextlib import ExitStack

import concourse.bacc as bacc
import concourse.tile as tile
from concourse import mybir
from concourse.bass_utils import run_bass_kernel_spmd

HIDDEN = 2048
T = 2048
N_HEADS = 16
N_KV = 4
HD = 128
THETA = 500000.0
NCORES = 8
SCALE = HD ** -0.5

F32 = mybir.dt.float32
F32R = mybir.dt.float32r
BF16 = mybir.dt.bfloat16
I32 = mybir.dt.int32

# bf16 for the qkv-projection inputs (hidden_states^T and w_qkv slice):
# halves the dominant input DMA traffic; the rest of the pipeline stays f32r.
BF16_IN = False
IN_DT = BF16 if BF16_IN else F32R

# within-head column permutation: evens then odds (so interleaved rope pairs
# become two contiguous partition halves in feature-major layout)
PERM = np.concatenate([np.arange(0, HD, 2), np.arange(1, HD, 2)])
# pair index p (0..63): p<44: even->pos row 1 (h), odd->row 2 (w); p>=44: row 0 (t)
ROW_MAP = np.array([(1 if p % 2 == 0 else 2) if p < 44 else 0 for p in range(64)])
INVF = (THETA ** (-(np.arange(64, dtype=np.float64) / 64))).astype(np.float32)

NT = T // 128      # 16 token tiles
NG = T // 512      # 4 token chunks
NH_T = HIDDEN // 128  # 16 hidden tiles


def _build(dbg=False):
    nc = bacc.Bacc("TRN2", target_bir_lowering=False, debug=False)
    d_xT = nc.dram_tensor("xT", [HIDDEN, T], IN_DT, kind="ExternalInput").ap()
    d_w = nc.dram_tensor("w_slice", [HIDDEN, 512], IN_DT, kind="ExternalInput").ap()
    d_wo = nc.dram_tensor("wo_slice", [256, HIDDEN], F32R, kind="ExternalInput").ap()
    d_pos = nc.dram_tensor("pos_sel", [128, T], I32, kind="ExternalInput").ap()
    d_invf = nc.dram_tensor("invf", [128, 1], F32, kind="ExternalInput").ap()
    d_svec = nc.dram_tensor("svec", [128, 1], F32, kind="ExternalInput").ap()
    d_mL = nc.dram_tensor("mask_l", [128, 128], F32R, kind="ExternalInput").ap()
    d_mR = nc.dram_tensor("mask_r", [128, 4, 512], F32R, kind="ExternalInput").ap()
    d_ident = nc.dram_tensor("ident", [128, 128], F32R, kind="ExternalInput").ap()
    d_ones = nc.dram_tensor("ones", [128, 128], F32R, kind="ExternalInput").ap()
    d_yT = nc.dram_tensor("yT", [HIDDEN, T], F32, kind="ExternalOutput").ap()
    if dbg:
        d_qkv = nc.dram_tensor("dbg_qkv", [128, 4, T], F32, kind="ExternalOutput").ap()
        d_cs = nc.dram_tensor("dbg_cs", [128, 2, T], F32, kind="ExternalOutput").ap()
        d_V = nc.dram_tensor("dbg_V", [128, NT, 128], F32, kind="ExternalOutput").ap()
        d_O = nc.dram_tensor("dbg_O", [128, 2, T], F32, kind="ExternalOutput").ap()

    TWO_PI = float(2 * np.pi)
    with tile.TileContext(nc) as tc, ExitStack() as ctx:
        const = ctx.enter_context(tc.tile_pool(name="const", bufs=1))
        big = ctx.enter_context(tc.tile_pool(name="big", bufs=1))

        # resident tiles
        w_sb = const.tile([128, NH_T, 512], IN_DT)      # qkv weight slice
        wo_sb = const.tile([128, 2, HIDDEN], F32R)      # o_proj rows
        mL_sb = const.tile([128, 128], F32R)            # causal mask, left factor
        mR_sb = const.tile([128, 4, 512], F32R)         # causal mask, right factor
        ident_sb = const.tile([128, 128], F32R)
        ones_sb = const.tile([128, 128], F32R)
        invf_sb = const.tile([128, 1], F32)
        svec_sb = const.tile([128, 1], F32)
        qkv_sb = big.tile([128, 4, T], F32R)            # q0|q1|k|v feature-major
        V_sb = big.tile([128, NT, 128], F32R)           # V token-major
        O_sb = big.tile([128, 2, T], F32R)              # attention out, feature-major
        cdup = big.tile([128, T], F32R)                 # cos table (dup halves)
        sflip = big.tile([128, T], F32R)                # sin table ([-s; s])

        nc.sync.dma_start(out=invf_sb, in_=d_invf)
        nc.sync.dma_start(out=svec_sb, in_=d_svec)

        # PSUM budget (8 banks): qkv accum 4 + shared(scores/V-transpose/o_proj) 2
        # + AV accum 1 + ones accum 1.
        xtp = ctx.enter_context(tc.tile_pool(name="xt", bufs=3))
        qkvp = ctx.enter_context(tc.tile_pool(name="qkvp", bufs=4, space="PSUM"))
        spp = ctx.enter_context(tc.tile_pool(name="spp", bufs=3, space="PSUM"))
        avp = ctx.enter_context(tc.tile_pool(name="avp", bufs=1, space="PSUM"))
        tbl = ctx.enter_context(tc.tile_pool(name="tbl", bufs=1))
        rp = ctx.enter_context(tc.tile_pool(name="rope", bufs=2))
        ep = ctx.enter_context(tc.tile_pool(name="ep", bufs=6))
        rv = ctx.enter_context(tc.tile_pool(name="rv", bufs=2))
        racc = ctx.enter_context(tc.tile_pool(name="racc", bufs=2))
        yo = ctx.enter_context(tc.tile_pool(name="yo", bufs=4))

        # ---- rope tables (all chunks upfront: keeps Sin/Exp from thrashing the
        # ACT function table). invf is pre-divided by 2pi on host:
        # turns = pos * invf; reduce to [-0.5, 0.5] turns via rne f32->i32
        # roundtrip; Sin with 2pi (and per-half sign) folded into ACT scale.
        #   cdup = cos(ang) both halves; sflip = [-sin; +sin] (svec = +-2pi)
        for gt in range(NG):
            tsl2 = np.s_[512 * gt:512 * (gt + 1)]
            pos_c = tbl.tile([128, 512], I32, tag="pos", name=f"pos{gt}")
            turns = tbl.tile([128, 512], F32, tag="turns", name=f"turns{gt}")
            turns_c = tbl.tile([128, 512], F32, tag="turnsc", name=f"turnsc{gt}")
            tint = tbl.tile([128, 512], I32, tag="ti", name=f"ti{gt}")
            tflt = tbl.tile([128, 512], F32, tag="tf", name=f"tf{gt}")
            nc.sync.dma_start(out=pos_c, in_=d_pos[:, tsl2])
            nc.gpsimd.tensor_copy(turns[:], pos_c[:])      # int32 -> fp32
            nc.vector.tensor_scalar_mul(turns[:], turns[:], invf_sb[:, 0:1])
            nc.vector.tensor_scalar_add(turns_c[:], turns[:], 0.25)
            nc.gpsimd.tensor_copy(tint[:], turns[:])       # round to nearest
            nc.gpsimd.tensor_copy(tflt[:], tint[:])
            nc.vector.tensor_sub(turns[:], turns[:], tflt[:])
            nc.scalar.activation(sflip[:, tsl2], turns[:],
                                 mybir.ActivationFunctionType.Sin,
                                 bias=0.0, scale=svec_sb[:, 0:1])
            # cos path: +0.25 turns offset (cos x = sin(x + pi/2))
            nc.gpsimd.tensor_copy(tint[:], turns_c[:])
            nc.gpsimd.tensor_copy(tflt[:], tint[:])
            nc.vector.tensor_sub(turns_c[:], turns_c[:], tflt[:])
            nc.scalar.activation(cdup[:, tsl2], turns_c[:],
                                 mybir.ActivationFunctionType.Sin,
                                 bias=0.0, scale=TWO_PI)

        def attn_and_oproj(g):
            tsl = np.s_[512 * g:512 * (g + 1)]
            if g == 0:
                nc.sync.dma_start(out=ones_sb, in_=d_ones)
                nc.sync.dma_start(out=mL_sb, in_=d_mL)
                nc.sync.dma_start(out=mR_sb, in_=d_mR)
                for h in range(2):
                    nc.sync.dma_start(out=wo_sb[:, h, :],
                                      in_=d_wo[128 * h:128 * (h + 1), :])
            # attention: scoresT -> exp -> AV accumulate (PE) + row-sum
            # accumulate (Pool, elementwise) -> one ones-matmul -> normalize
            for h in range(2):
                qc = qkv_sb[:, h, tsl]
                po = avp.tile([128, 512], F32, tag="av", name=f"po{g}_{h}")
                ra = racc.tile([128, 512], F32R, tag="ra", name=f"ra{g}_{h}")
                rb = racc.tile([128, 512], F32R, tag="rb", name=f"rb{g}_{h}")
                jmax = 4 * g + 4
                for j in range(jmax):
                    m = j - 4 * g
                    ps = spp.tile([128, 512], F32, tag="sp", name=f"s{g}_{h}_{j}")
                    nc.tensor.matmul(ps[:], qkv_sb[:, 2, 128 * j:128 * (j + 1)], qc,
                                     start=True, stop=(m < 0))
                    if m >= 0:
                        # additive causal mask (-1e9 on invalid) via rank-
                        # factored matmul accumulated into the scores psum
                        nc.tensor.matmul(ps[:], mL_sb[:], mR_sb[:, m, :],
                                         start=False, stop=True)
                    E = ep.tile([128, 512], F32R, tag="e", name=f"e{g}_{h}_{j}")
                    nc.scalar.activation(E[:], ps[:],
                                         mybir.ActivationFunctionType.Exp,
                                         scale=SCALE)
                    nc.tensor.matmul(po[:], V_sb[:, j, :], E[:],
                                     start=(j == 0), stop=(j == jmax - 1))
                    # row-sum partials: even j on DVE, odd j on Pool
                    if j == 0:
                        nc.gpsimd.tensor_copy(ra[:], E[:])
                    elif j == 1:
                        nc.gpsimd.tensor_copy(rb[:], E[:])
                    elif j % 2 == 0:
                        nc.vector.tensor_add(ra[:], ra[:], E[:])
                    else:
                        nc.gpsimd.tensor_add(rb[:], rb[:], E[:])
                nc.vector.tensor_add(ra[:], ra[:], rb[:])
                # r broadcast across partitions via one all-ones matmul
                pr = spp.tile([128, 512], F32, tag="sp", name=f"pr{g}_{h}")
                nc.tensor.matmul(pr[:], ones_sb[:], ra[:], start=True, stop=True)
                rinv = rv.tile([128, 512], F32, tag="rv", name=f"rinv{g}_{h}")
                nc.vector.reciprocal(rinv[:], pr[:])
                nc.vector.tensor_mul(O_sb[:, h, tsl], po[:], rinv[:])

            # o_proj partial chunk: yT[:, tsl] = sum_h wo_h.T @ O_h
            for i in range(NH_T):
                py = spp.tile([128, 512], F32, tag="sp", name=f"y{g}_{i}")
                for h in range(2):
                    nc.tensor.matmul(py[:], wo_sb[:, h, 128 * i:128 * (i + 1)],
                                     O_sb[:, h, tsl], start=(h == 0), stop=(h == 1))
                yt = yo.tile([128, 512], F32, tag="yo", name=f"yt{g}_{i}")
                if i % 2 == 1:
                    nc.scalar.copy(yt[:], py[:])
                else:
                    nc.vector.tensor_copy(yt[:], py[:])
                nc.sync.dma_start(out=d_yT[128 * i:128 * (i + 1), tsl], in_=yt[:])

        # main loop, software-pipelined one chunk deep: while chunk g's
        # projection + rope run (DMA/DVE-heavy), the PE executes chunk g-1's
        # attention + o_proj.
        for g in range(NG):
            tsl = np.s_[512 * g:512 * (g + 1)]

            # ---- qkv projection chunk, feature-major: qkv[f, t] = w.T @ xT
            # xT/w loads are batched 4 hidden-tiles per DMA (1MB transfers):
            # the DMA sequencer's per-descriptor processing is near-critical
            psums = [qkvp.tile([128, 512], F32, tag="qkvps", name=f"qkvps_{g}_{i}")
                     for i in range(4)]
            for hb in range(NH_T // 4):
                if g == 0:
                    nc.sync.dma_start(
                        out=w_sb[:, 4 * hb:4 * (hb + 1), :],
                        in_=d_w[512 * hb:512 * (hb + 1), :].rearrange(
                            "(a p) c -> p a c", p=128))
                xt_b = xtp.tile([128, 4, 512], IN_DT, tag="xt", name=f"xt_{g}_{hb}")
                nc.sync.dma_start(
                    out=xt_b,
                    in_=d_xT[512 * hb:512 * (hb + 1), tsl].rearrange(
                        "(a p) c -> p a c", p=128))
                for k in range(4):
                    h = 4 * hb + k
                    for i in range(4):
                        nc.tensor.matmul(
                            psums[i][:], w_sb[:, h, 128 * i:128 * (i + 1)],
                            xt_b[:, k, :],
                            start=(h == 0), stop=(h == NH_T - 1))
            for i in (2, 0, 1, 3):   # k first: it gates rope -> next-chunk scores
                nc.vector.tensor_copy(qkv_sb[:, i, tsl], psums[i][:])

            if g == 0:
                # ident is needed by this chunk's V-transposes; the other
                # constants load inside attn_and_oproj(0), off the front path
                nc.sync.dma_start(out=ident_sb, in_=d_ident)
            # ---- rope chunk (k first: it unblocks the next chunk's scores)
            for t3 in (2, 0, 1):
                x = qkv_sb[:, t3, tsl]
                xs = rp.tile([128, 512], F32R, tag="xs", name=f"xs{g}_{t3}")
                nc.sync.dma_start(out=xs[0:64, :], in_=x[64:128, :])
                nc.sync.dma_start(out=xs[64:128, :], in_=x[0:64, :])
                t1 = rp.tile([128, 512], F32R, tag="t1", name=f"t1_{g}_{t3}")
                t2 = rp.tile([128, 512], F32R, tag="t2", name=f"t2_{g}_{t3}")
                nc.vector.tensor_mul(t1[:], x, cdup[:, tsl])
                nc.vector.tensor_mul(t2[:], xs[:], sflip[:, tsl])
                nc.vector.tensor_add(x, t1[:], t2[:])

            # ---- V transposes for this chunk (shared psum slots, tag "sp")
            for j in range(4 * g, 4 * g + 4):
                pt = spp.tile([128, 512], F32, tag="sp", name=f"vt{j}")
                nc.tensor.transpose(pt[:, 0:128].bitcast(F32R),
                                    qkv_sb[:, 3, 128 * j:128 * (j + 1)], ident_sb[:])
                nc.vector.tensor_copy(V_sb[:, j, :], pt[:, 0:128].bitcast(F32R))

            if g > 0:
                attn_and_oproj(g - 1)
        attn_and_oproj(NG - 1)

        if dbg:
            nc.sync.dma_start(out=d_qkv, in_=qkv_sb[:].bitcast(F32))
            nc.sync.dma_start(out=d_cs[:, 0, :], in_=cdup[:].bitcast(F32))
            nc.sync.dma_start(out=d_cs[:, 1, :], in_=sflip[:].bitcast(F32))
            nc.sync.dma_start(out=d_V, in_=V_sb[:].bitcast(F32))
            nc.sync.dma_start(out=d_O, in_=O_sb[:].bitcast(F32))

    nc.compile()
    return nc


_NC_CACHE = None


def _get_nc():
    global _NC_CACHE
    if _NC_CACHE is None:
        _NC_CACHE = _build()
    return _NC_CACHE


def _host_prep(positions, hidden_states, w_qkv, w_o):
    positions = np.asarray(positions, dtype=np.int32)
    hidden_states = np.asarray(hidden_states, dtype=np.float32)
    w_qkv = np.asarray(w_qkv, dtype=np.float32)
    w_o = np.asarray(w_o, dtype=np.float32)

    in_np = ml_dtypes.bfloat16 if BF16_IN else np.float32
    xT = np.ascontiguousarray(hidden_states.T).astype(in_np)
    pos_sel = np.ascontiguousarray(positions[np.concatenate([ROW_MAP, ROW_MAP])])
    invf = np.ascontiguousarray(
        (np.concatenate([INVF, INVF]) / (2 * np.pi)).astype(np.float32).reshape(128, 1))
    tp = np.float32(2 * np.pi)
    svec = np.concatenate([-tp * np.ones(64, np.float32),
                           tp * np.ones(64, np.float32)]).reshape(128, 1)
    # additive causal mask factors: invalid(dk, dq) = [dq - 128m + 1 <= dk]
    #   = sum_p L[p, dk] * Rm[p, dq],  L[p, dk] = [p <= dk],
    #   Rm[p, dq] = [p == max(dq - 128m + 1, 0)]  (scaled by -1e9)
    mask_l = (np.arange(128)[:, None] <= np.arange(128)[None, :]).astype(np.float32)
    mask_r = np.zeros((128, 4, 512), dtype=np.float32)
    for m in range(4):
        c = np.maximum(np.arange(512) - 128 * m + 1, 0)
        valid_rows = c <= 127
        mask_r[c[valid_rows], m, np.arange(512)[valid_rows]] = -1e9
    # sanity: factored mask == boolean causal mask
    dq = np.arange(512)[None, :]
    dk = np.arange(128)[:, None]
    for m in range(4):
        got = mask_l.T @ mask_r[:, m, :]
        want = np.where(dq < dk + 128 * m, -1e9, 0.0)
        assert np.array_equal(got, want), f"mask factorization wrong for m={m}"
    ident = np.eye(128, dtype=np.float32)
    ones = np.ones((128, 128), dtype=np.float32)

    q_size = N_HEADS * HD
    kv_size = N_KV * HD
    in_maps = []
    for c in range(NCORES):
        cols = [w_qkv[:, 2 * c * HD + PERM], w_qkv[:, (2 * c + 1) * HD + PERM]]
        kc = c // 2
        cols.append(w_qkv[:, q_size + kc * HD + PERM])
        cols.append(w_qkv[:, q_size + kv_size + kc * HD:q_size + kv_size + (kc + 1) * HD])
        w_slice = np.ascontiguousarray(np.concatenate(cols, axis=1)).astype(in_np)
        wo_slice = np.ascontiguousarray(w_o[2 * c * HD:(2 * c + 2) * HD])
        in_maps.append({
            "xT": xT, "w_slice": w_slice, "wo_slice": wo_slice,
            "pos_sel": pos_sel, "invf": invf, "svec": svec,
            "mask_l": mask_l, "mask_r": mask_r, "ident": ident, "ones": ones,
        })
    return in_maps


def kernel(positions, hidden_states, w_qkv, w_o):
    nc = _get_nc()
    in_maps = _host_prep(positions, hidden_states, w_qkv, w_o)
    # one retry: transient NRT/device errors (e.g. NRT_EXEC_UNIT_UNRECOVERABLE
    # from a wedged core) were observed to succeed on re-dispatch
    try:
        res = run_bass_kernel_spmd(nc, in_maps, core_ids=list(range(NCORES)))
    except Exception:
        import time
        time.sleep(2.0)
        res = run_bass_kernel_spmd(nc, in_maps, core_ids=list(range(NCORES)))
    yT = np.zeros((HIDDEN, T), dtype=np.float64)
    for c in range(NCORES):
        yT += res.results[c]["yT"]
    return np.ascontiguousarray(yT.T).astype(np.float32)



# revision 2
# speedup vs baseline: 1.2039x; 1.2039x over previous
"""Trainium2 Bass kernel for Ernie4.5-VL attention (mRoPE + GQA causal attention).

Sharding: tensor-parallel over heads across 8 cores. Each core computes
2 q heads + its kv head (replicated per core pair): qkv projection
(q/k feature-major, V token-major directly — no transposes), interleaved
mRoPE (via a host-side even/odd column permutation of the q/k weight
slices so the rotation becomes two contiguous partition halves), causal
attention with unnormalized softmax (denominator via bf16 tile adds +
one all-ones matmul), and the o_proj partial product. Host sums the 8
partial outputs.

All tensors move through SBUF/DRAM as bf16 (halves DMA + enables DVE
fast modes); matmuls are bf16 in / fp32 psum out; psum evacuations round
once to bf16. The attention inner loop runs scores one step ahead of AV
so the exp (ACT) latency is hidden behind the next scores matmul.
"""
import numpy as np
import ml_dtypes
from contextlib import ExitStack

import concourse.bacc as bacc
import concourse.tile as tile
from concourse import mybir
from concourse.bass_utils import run_bass_kernel_spmd

HIDDEN = 2048
T = 2048
N_HEADS = 16
N_KV = 4
HD = 128
THETA = 500000.0
NCORES = 8
SCALE = HD ** -0.5

F32 = mybir.dt.float32
BF16 = mybir.dt.bfloat16
I32 = mybir.dt.int32

# within-head column permutation: evens then odds (so interleaved rope pairs
# become two contiguous partition halves in feature-major layout)
PERM = np.concatenate([np.arange(0, HD, 2), np.arange(1, HD, 2)])
# pair index p (0..63): p<44: even->pos row 1 (h), odd->row 2 (w); p>=44: row 0 (t)
ROW_MAP = np.array([(1 if p % 2 == 0 else 2) if p < 44 else 0 for p in range(64)])
INVF = (THETA ** (-(np.arange(64, dtype=np.float64) / 64))).astype(np.float32)

NT = T // 128      # 16 token tiles
NG = T // 512      # 4 token chunks
NH_T = HIDDEN // 128  # 16 hidden tiles


def _build(dbg=False):
    nc = bacc.Bacc("TRN2", target_bir_lowering=False, debug=False)
    d_xT = nc.dram_tensor("xT", [HIDDEN, T], BF16, kind="ExternalInput").ap()
    d_w = nc.dram_tensor("w_slice", [HIDDEN, 512], BF16, kind="ExternalInput").ap()
    d_wo = nc.dram_tensor("wo_slice", [256, HIDDEN], BF16, kind="ExternalInput").ap()
    d_pos = nc.dram_tensor("pos_sel", [128, T], I32, kind="ExternalInput").ap()
    d_invf = nc.dram_tensor("invf", [128, 1], F32, kind="ExternalInput").ap()
    d_svec = nc.dram_tensor("svec", [128, 1], F32, kind="ExternalInput").ap()
    d_mL = nc.dram_tensor("mask_l", [128, 128], BF16, kind="ExternalInput").ap()
    d_mR = nc.dram_tensor("mask_r", [128, 4, 512], BF16, kind="ExternalInput").ap()
    d_ones = nc.dram_tensor("ones", [128, 128], BF16, kind="ExternalInput").ap()
    d_yT = nc.dram_tensor("yT", [HIDDEN, T], BF16, kind="ExternalOutput").ap()
    if dbg:
        d_qkv = nc.dram_tensor("dbg_qkv", [128, 3, T], F32, kind="ExternalOutput").ap()
        d_cs = nc.dram_tensor("dbg_cs", [128, 2, T], F32, kind="ExternalOutput").ap()
        d_V = nc.dram_tensor("dbg_V", [128, NT, 128], F32, kind="ExternalOutput").ap()
        d_O = nc.dram_tensor("dbg_O", [128, 2, T], F32, kind="ExternalOutput").ap()

    TWO_PI = float(2 * np.pi)
    with tile.TileContext(nc) as tc, ExitStack() as ctx:
        const = ctx.enter_context(tc.tile_pool(name="const", bufs=1))
        big = ctx.enter_context(tc.tile_pool(name="big", bufs=1))

        # resident tiles
        w_sb = const.tile([128, NH_T, 512], BF16)       # qkv weight slice
        wo_sb = const.tile([128, 2, HIDDEN], BF16)      # o_proj rows
        mL_sb = const.tile([128, 128], BF16)            # causal mask, left factor
        mR_sb = const.tile([128, 4, 512], BF16)         # causal mask, right factor
        ones_sb = const.tile([128, 128], BF16)
        invf_sb = const.tile([128, 1], F32)
        svec_sb = const.tile([128, 1], F32)
        pos_sb = const.tile([128, T], I32)
        qkv_sb = big.tile([128, 3, T], BF16)            # q0|q1|k feature-major
        V_sb = big.tile([128, NT, 128], BF16)           # V token-major
        O_sb = big.tile([128, 2, T], BF16)              # attention out, feature-major
        cdup = big.tile([128, T], BF16)                 # cos table (dup halves)
        sflip = big.tile([128, T], BF16)                # sin table ([-s; s])

        nc.sync.dma_start(out=invf_sb, in_=d_invf)
        nc.sync.dma_start(out=svec_sb, in_=d_svec)
        nc.sync.dma_start(out=pos_sb, in_=d_pos)

        # PSUM budget (8 banks): q0/q1/k accum 3 + V-direct 1 +
        # shared(scores/o_proj) 3 + AV accum 1.
        xtp = ctx.enter_context(tc.tile_pool(name="xt", bufs=2))
        qkvp = ctx.enter_context(tc.tile_pool(name="qkvp", bufs=3, space="PSUM"))
        vdp = ctx.enter_context(tc.tile_pool(name="vdp", bufs=1, space="PSUM"))
        spp = ctx.enter_context(tc.tile_pool(name="spp", bufs=3, space="PSUM"))
        avp = ctx.enter_context(tc.tile_pool(name="avp", bufs=1, space="PSUM"))
        tbl = ctx.enter_context(tc.tile_pool(name="tbl", bufs=1))
        rp = ctx.enter_context(tc.tile_pool(name="rope", bufs=2))
        ep = ctx.enter_context(tc.tile_pool(name="ep", bufs=6))
        rv = ctx.enter_context(tc.tile_pool(name="rv", bufs=2))
        racc = ctx.enter_context(tc.tile_pool(name="racc", bufs=2))
        yo = ctx.enter_context(tc.tile_pool(name="yo", bufs=2))

        # ---- rope tables (all chunks upfront: keeps Sin/Exp from thrashing the
        # ACT function table). invf is pre-divided by 2pi on host:
        # turns = pos * invf; reduce to [-0.5, 0.5] turns via rne f32->i32
        # roundtrip; Sin with 2pi (and per-half sign) folded into ACT scale.
        #   cdup = cos(ang) both halves; sflip = [-sin; +sin] (svec = +-2pi)
        for gt in range(NG):
            tsl2 = np.s_[512 * gt:512 * (gt + 1)]
            turns = tbl.tile([128, 512], F32, tag="turns", name=f"turns{gt}")
            turns_c = tbl.tile([128, 512], F32, tag="turnsc", name=f"turnsc{gt}")
            tint = tbl.tile([128, 512], I32, tag="ti", name=f"ti{gt}")
            tflt = tbl.tile([128, 512], F32, tag="tf", name=f"tf{gt}")
            nc.gpsimd.tensor_copy(turns[:], pos_sb[:, tsl2])   # int32 -> fp32
            nc.vector.tensor_scalar_mul(turns[:], turns[:], invf_sb[:, 0:1])
            nc.vector.tensor_scalar_add(turns_c[:], turns[:], 0.25)
            nc.gpsimd.tensor_copy(tint[:], turns[:])       # round to nearest
            nc.gpsimd.tensor_copy(tflt[:], tint[:])
            nc.vector.tensor_sub(turns[:], turns[:], tflt[:])
            nc.scalar.activation(sflip[:, tsl2], turns[:],
                                 mybir.ActivationFunctionType.Sin,
                                 bias=0.0, scale=svec_sb[:, 0:1])
            # cos path: +0.25 turns offset (cos x = sin(x + pi/2))
            nc.gpsimd.tensor_copy(tint[:], turns_c[:])
            nc.gpsimd.tensor_copy(tflt[:], tint[:])
            nc.vector.tensor_sub(turns_c[:], turns_c[:], tflt[:])
            nc.scalar.activation(cdup[:, tsl2], turns_c[:],
                                 mybir.ActivationFunctionType.Sin,
                                 bias=0.0, scale=TWO_PI)

        def attn_and_oproj(g):
            tsl = np.s_[512 * g:512 * (g + 1)]
            # attention: scoresT -> exp -> AV accumulate, scores emitted one
            # j-step ahead of AV so exp latency hides behind the next matmul
            for h in range(2):
                qc = qkv_sb[:, h, tsl]
                po = avp.tile([128, 512], F32, tag="av", name=f"po{g}_{h}")
                ra = racc.tile([128, 512], BF16, tag="ra", name=f"ra{g}_{h}")
                rb = racc.tile([128, 512], BF16, tag="rb", name=f"rb{g}_{h}")
                jmax = 4 * g + 4
                Es = [None] * jmax
                ps_prev = None
                for j in range(jmax):
                    m = j - 4 * g
                    ps = spp.tile([128, 512], F32, tag="sp", name=f"s{g}_{h}_{j}")
                    nc.tensor.matmul(ps[:], qkv_sb[:, 2, 128 * j:128 * (j + 1)], qc,
                                     start=True, stop=(m < 0))
                    if m >= 0:
                        # additive causal mask (-1e9 on invalid) via rank-
                        # factored matmul accumulated into the scores psum
                        nc.tensor.matmul(ps[:], mL_sb[:], mR_sb[:, m, :],
                                         start=False, stop=True)
                    E = ep.tile([128, 512], BF16, tag="e", name=f"e{g}_{h}_{j}")
                    Es[j] = E
                    nc.scalar.activation(E[:], ps[:],
                                         mybir.ActivationFunctionType.Exp,
                                         scale=SCALE)
                    # row-sum partials: two bf16 accumulators on DVE
                    if j == 0:
                        nc.vector.tensor_copy(ra[:], E[:])
                    elif j == 1:
                        nc.vector.tensor_copy(rb[:], E[:])
                    elif j % 2 == 0:
                        nc.vector.tensor_add(ra[:], ra[:], E[:])
                    else:
                        nc.vector.tensor_add(rb[:], rb[:], E[:])
                    if j >= 1:
                        nc.tensor.matmul(po[:], V_sb[:, j - 1, :], Es[j - 1][:],
                                         start=(j == 1), stop=False)
                nc.tensor.matmul(po[:], V_sb[:, jmax - 1, :], Es[jmax - 1][:],
                                 start=(jmax == 1), stop=True)
                nc.vector.tensor_add(ra[:], ra[:], rb[:])
                # r broadcast across partitions via one all-ones matmul
                pr = spp.tile([128, 512], F32, tag="sp", name=f"pr{g}_{h}")
                nc.tensor.matmul(pr[:], ones_sb[:], ra[:], start=True, stop=True)
                rinv = rv.tile([128, 512], F32, tag="rv", name=f"rinv{g}_{h}")
                nc.vector.reciprocal(rinv[:], pr[:])
                nc.vector.tensor_mul(O_sb[:, h, tsl], po[:], rinv[:])

            # o_proj partial chunk: yT[:, tsl] = sum_h wo_h.T @ O_h
            ybuf = yo.tile([128, NH_T, 512], BF16, tag="yo", name=f"yb{g}")
            for i in range(NH_T):
                py = spp.tile([128, 512], F32, tag="sp", name=f"y{g}_{i}")
                for h in range(2):
                    nc.tensor.matmul(py[:], wo_sb[:, h, 128 * i:128 * (i + 1)],
                                     O_sb[:, h, tsl], start=(h == 0), stop=(h == 1))
                if i % 2 == 1:
                    nc.scalar.copy(ybuf[:, i, :], py[:])
                else:
                    nc.vector.tensor_copy(ybuf[:, i, :], py[:])
            nc.sync.dma_start(
                out=d_yT[:, tsl].rearrange("(a p) c -> p a c", p=128),
                in_=ybuf)

        # main loop, software-pipelined one chunk deep: PE alternates
        # [proj g | attn g-1]; DVE/ACT do evac+rope g and attn g-1's
        # exp/rowsum work in the shadow of the other phase.
        for g in range(NG):
            tsl = np.s_[512 * g:512 * (g + 1)]

            # ---- xt chunk DMA (chunk 0 split 4-ways for fast startup)
            xt_b = xtp.tile([128, NH_T, 512], BF16, tag="xt", name=f"xt_{g}")
            if g == 0:
                for q4 in range(4):
                    nc.sync.dma_start(
                        out=xt_b[:, 4 * q4:4 * (q4 + 1), :],
                        in_=d_xT[512 * q4:512 * (q4 + 1), tsl].rearrange(
                            "(a p) c -> p a c", p=128))
                    nc.sync.dma_start(
                        out=w_sb[:, 4 * q4:4 * (q4 + 1), :],
                        in_=d_w[512 * q4:512 * (q4 + 1), :].rearrange(
                            "(a p) c -> p a c", p=128))
                nc.sync.dma_start(out=mL_sb, in_=d_mL)
                nc.sync.dma_start(out=mR_sb, in_=d_mR)
                nc.sync.dma_start(out=ones_sb, in_=d_ones)
                nc.sync.dma_start(
                    out=wo_sb,
                    in_=d_wo.rearrange("(a p) c -> p a c", p=128))
            else:
                nc.sync.dma_start(
                    out=xt_b,
                    in_=d_xT[:, tsl].rearrange("(a p) c -> p a c", p=128))

            # ---- qkv projection chunk: q0/q1/k feature-major psums,
            # V token-major directly (lhsT = xt token tile, rhs = w_v)
            psums = [qkvp.tile([128, 512], F32, tag="qkvps", name=f"qkvps_{g}_{i}")
                     for i in range(3)]
            vd = vdp.tile([128, 4, 128], F32, tag="vd", name=f"vd_{g}")
            for hb in range(NH_T):
                for i in range(3):
                    nc.tensor.matmul(
                        psums[i][:], w_sb[:, hb, 128 * i:128 * (i + 1)],
                        xt_b[:, hb, :],
                        start=(hb == 0), stop=(hb == NH_T - 1))
                for tt in range(4):
                    nc.tensor.matmul(
                        vd[:, tt, :], xt_b[:, hb, 128 * tt:128 * (tt + 1)],
                        w_sb[:, hb, 384:512],
                        start=(hb == 0), stop=(hb == NH_T - 1))

            # ---- evac + rope (k first: it gates this chunk's scores).
            # swap halves via SBUF->SBUF DMA; t1/t2 in f32 so qkv_sb rounds
            # to bf16 once. t2-mul on Pool, t1-mul + add on DVE.
            xs = rp.tile([128, 3, 512], BF16, tag="xs", name=f"xs{g}")

            def rope_one(t3):
                x = qkv_sb[:, t3, tsl]
                nc.vector.tensor_copy(x, psums[t3][:])
                nc.sync.dma_start(out=xs[0:64, t3, :], in_=x[64:128, :])
                nc.sync.dma_start(out=xs[64:128, t3, :], in_=x[0:64, :])
                t1 = rp.tile([128, 512], F32, tag="t1", name=f"t1_{g}_{t3}")
                t2 = rp.tile([128, 512], F32, tag="t2", name=f"t2_{g}_{t3}")
                nc.vector.tensor_mul(t1[:], x, cdup[:, tsl])
                nc.gpsimd.tensor_mul(t2[:], xs[:, t3, :], sflip[:, tsl])
                nc.vector.tensor_add(x, t1[:], t2[:])

            rope_one(2)
            for tt in range(4):
                nc.gpsimd.tensor_copy(V_sb[:, 4 * g + tt, :], vd[:, tt, :])
            rope_one(0)
            rope_one(1)

            if g > 0:
                attn_and_oproj(g - 1)
        attn_and_oproj(NG - 1)

        if dbg:
            nc.sync.dma_start(out=d_qkv, in_=qkv_sb[:].bitcast(F32))
            nc.sync.dma_start(out=d_cs[:, 0, :], in_=cdup[:].bitcast(F32))
            nc.sync.dma_start(out=d_cs[:, 1, :], in_=sflip[:].bitcast(F32))
            nc.sync.dma_start(out=d_V, in_=V_sb[:].bitcast(F32))
            nc.sync.dma_start(out=d_O, in_=O_sb[:].bitcast(F32))

    nc.compile()
    return nc


_NC_CACHE = None


def _get_nc():
    global _NC_CACHE
    if _NC_CACHE is None:
        _NC_CACHE = _build()
    return _NC_CACHE


def _host_prep(positions, hidden_states, w_qkv, w_o):
    positions = np.asarray(positions, dtype=np.int32)
    hidden_states = np.asarray(hidden_states, dtype=np.float32)
    w_qkv = np.asarray(w_qkv, dtype=np.float32)
    w_o = np.asarray(w_o, dtype=np.float32)

    bf = ml_dtypes.bfloat16
    xT = np.ascontiguousarray(hidden_states.T).astype(bf)
    pos_sel = np.ascontiguousarray(positions[np.concatenate([ROW_MAP, ROW_MAP])])
    invf = np.ascontiguousarray(
        (np.concatenate([INVF, INVF]) / (2 * np.pi)).astype(np.float32).reshape(128, 1))
    tp = np.float32(2 * np.pi)
    svec = np.concatenate([-tp * np.ones(64, np.float32),
                           tp * np.ones(64, np.float32)]).reshape(128, 1)
    # additive causal mask factors: invalid(dk, dq) = [dq - 128m + 1 <= dk]
    #   = sum_p L[p, dk] * Rm[p, dq],  L[p, dk] = [p <= dk],
    #   Rm[p, dq] = [p == max(dq - 128m + 1, 0)]  (scaled by -1e9)
    mask_l = (np.arange(128)[:, None] <= np.arange(128)[None, :]).astype(np.float32)
    mask_r = np.zeros((128, 4, 512), dtype=np.float32)
    for m in range(4):
        c = np.maximum(np.arange(512) - 128 * m + 1, 0)
        valid_rows = c <= 127
        mask_r[c[valid_rows], m, np.arange(512)[valid_rows]] = -1e9
    # sanity: factored mask == boolean causal mask
    dq = np.arange(512)[None, :]
    dk = np.arange(128)[:, None]
    for m in range(4):
        got = mask_l.T @ mask_r[:, m, :]
        want = np.where(dq < dk + 128 * m, -1e9, 0.0)
        assert np.array_equal(got, want), f"mask factorization wrong for m={m}"
    ones = np.ones((128, 128), dtype=np.float32)

    q_size = N_HEADS * HD
    kv_size = N_KV * HD
    in_maps = []
    for c in range(NCORES):
        cols = [w_qkv[:, 2 * c * HD + PERM], w_qkv[:, (2 * c + 1) * HD + PERM]]
        kc = c // 2
        cols.append(w_qkv[:, q_size + kc * HD + PERM])
        cols.append(w_qkv[:, q_size + kv_size + kc * HD:q_size + kv_size + (kc + 1) * HD])
        w_slice = np.ascontiguousarray(np.concatenate(cols, axis=1)).astype(bf)
        wo_slice = np.ascontiguousarray(w_o[2 * c * HD:(2 * c + 2) * HD]).astype(bf)
        in_maps.append({
            "xT": xT, "w_slice": w_slice, "wo_slice": wo_slice,
            "pos_sel": pos_sel, "invf": invf, "svec": svec,
            "mask_l": mask_l.astype(bf), "mask_r": mask_r.astype(bf),
            "ones": ones.astype(bf),
        })
    return in_maps


def kernel(positions, hidden_states, w_qkv, w_o):
    nc = _get_nc()
    in_maps = _host_prep(positions, hidden_states, w_qkv, w_o)
    # one retry: transient NRT/device errors (e.g. NRT_EXEC_UNIT_UNRECOVERABLE
    # from a wedged core) were observed to succeed on re-dispatch
    try:
        res = run_bass_kernel_spmd(nc, in_maps, core_ids=list(range(NCORES)))
    except Exception:
        import time
        time.sleep(2.0)
        res = run_bass_kernel_spmd(nc, in_maps, core_ids=list(range(NCORES)))
    yT = np.zeros((HIDDEN, T), dtype=np.float64)
    for c in range(NCORES):
        yT += np.asarray(res.results[c]["yT"], dtype=np.float64)
    return np.ascontiguousarray(yT.T).astype(np.float32)
